# revision 48
# baseline (speedup 1.0000x reference)
"""Causal self-attention (B=4, T=2048, C=1024, 16 heads) on 8 Trainium2 cores.

Sharding: core c -> batch b = c//2 (4 data-parallel groups), head shard
s = c%2 (Megatron tensor-parallel: 8 of 16 heads, qkv column-sharded,
proj row-sharded).  Each core computes a partial projection output for
its batch; the host sums the two partials per batch (+ b_proj).

Matmul precision strategy (validated vs the fp32 reference):
  qkv GEMMs   fp8e4m3 DoubleRow with 3-term error compensation
              (xhi@Whi + xhi@Wlo + xlo@Whi); each DoubleRow instruction
              contracts 256 rows at 0.5 cycles/column.
  S = K^T Q   fp8e4m3 DoubleRow: Q^T/K^T are stored x32 in a
              [32-partition, 2-fold] layout so one instruction contracts
              the full d_head=64; exp scale absorbs the 1024x.
  Y = V P~    bf16 (fp8 here fails the accuracy budget).
  proj        bf16.

On-core layout is "feature-major" to avoid transposes:
  Q^T, K^T  [32, hh, dhi, tok] fp8 (x32)  from  W^T @ x^T  quarters
  V         [tok, h, 65] bf16 (ones col -> Z) from x^T-chunks as lhsT
  S^T       [k, q] blocks  = (K^T-chunk)^T @ Q^T-chunk  (DoubleRow d=64)
  P~^T      = exp(S^T * SCALE/1024)  (no max subtraction: |logit| < ~4)
  Y^T[h]    [65, q] = V-hat^T @ P~^T  (row 64 = Z = sum_k P~)
  out       [tok, C] = (Y^T/Z)^T @ W_proj-shard  (K=512 contraction)
"""

import numpy as np
import ml_dtypes
from contextlib import ExitStack

import concourse.bass as bass
import concourse.tile as tile
from concourse import mybir, bacc
from concourse.bass_utils import run_bass_kernel_spmd

F32 = mybir.dt.float32
F32R = mybir.dt.float32r
BF16 = mybir.dt.bfloat16
FP8 = mybir.dt.float8e4
AF = mybir.ActivationFunctionType
ALU = mybir.AluOpType
DR = mybir.MatmulPerfMode.DoubleRow
E4 = ml_dtypes.float8_e4m3

B, T, C = 4, 2048, 1024
NH, DH = 16, 64
SCALE = 1.0 / float(np.sqrt(DH))
NCORES = 8
HPC = 8              # heads per core
WCOLS = HPC * DH     # 512 qkv columns per core
NPAIR = HPC // 2     # head pairs (row/psum packing unit)
KC = T // 128        # 16 key-token chunks
QC = T // 512        # 4 query chunks
WS = 32.0            # fp8 weight scale (W stored x32; Q/K stored x32)


def _build_program(use_bias: bool):
    nc = bacc.Bacc(trn_type="TRN2", target_bir_lowering=False, debug=False)

    xhi_d = nc.dram_tensor("xhi", [C, T], FP8, kind="ExternalInput").ap()
    xlo_d = nc.dram_tensor("xlo", [C, T], FP8, kind="ExternalInput").ap()
    w_d = {}
    for tname in ("wq", "wk", "wv"):
        for half in ("hi", "lo"):
            w_d[tname + half] = nc.dram_tensor(
                tname + half, [C, WCOLS], FP8, kind="ExternalInput"
            ).ap()
    wp = nc.dram_tensor("wp", [WCOLS, C], BF16, kind="ExternalInput").ap()
    if use_bias:
        bq = nc.dram_tensor("bq", [128, 4], F32, kind="ExternalInput").ap()
        bk = nc.dram_tensor("bk", [128, 4], F32, kind="ExternalInput").ap()
        bv = nc.dram_tensor("bv", [WCOLS], F32, kind="ExternalInput").ap()
    out = nc.dram_tensor("out", [T, C], BF16, kind="ExternalOutput").ap()

    with tile.TileContext(nc) as tc, ExitStack() as ctx:
        pool = ctx.enter_context(tc.tile_pool(name="main", bufs=1))
        xpool = ctx.enter_context(tc.tile_pool(name="xt", bufs=2))
        ptpool = ctx.enter_context(tc.tile_pool(name="pt", bufs=12))
        zpool = ctx.enter_context(tc.tile_pool(name="zr", bufs=6))
        ytmpool = ctx.enter_context(tc.tile_pool(name="ytm", bufs=6))
        opool = ctx.enter_context(tc.tile_pool(name="out", bufs=4))
        ps_mm = ctx.enter_context(tc.tile_pool(name="ps_mm", bufs=2, space="PSUM"))
        ps_s = ctx.enter_context(tc.tile_pool(name="ps_s", bufs=2, space="PSUM"))
        ps_y = ctx.enter_context(tc.tile_pool(name="ps_y", bufs=1, space="PSUM"))

        # Q^T/K^T fp8 (x32) in the DoubleRow d-fold layout, per 512-tok slab:
        # tile[p, hh, dhi, t] = (x @ Wq)[col = 64*(4*hh + p//32) + 32*dhi + p%32, t]
        QTf = [pool.tile([128, 2, 2, 512], FP8, tag=f"qtf{i}", name=f"QTf{i}")
               for i in range(QC)]
        KTf = [pool.tile([128, 2, 2, 512], FP8, tag=f"ktf{i}", name=f"KTf{i}")
               for i in range(QC)]
        # V tiles head-major with a trailing ones column per head: [tok, h, 65]
        V = [pool.tile([128, HPC, DH + 1], BF16, tag=f"v{t}", name=f"v{t}") for t in range(KC)]
        for t in range(KC):
            nc.vector.memset(V[t][:, :, DH : DH + 1], 1.0)
        YT = [pool.tile([128, T], BF16, tag=f"yt{p}", name=f"yt{p}") for p in range(NPAIR)]
        # identity for PE-transpose of the q-major Y back to feature-major
        ident = pool.tile([128, 128], BF16, tag="ident", name="ident")
        nc.vector.memset(ident, 1.0)
        nc.gpsimd.affine_select(
            out=ident, in_=ident, compare_op=ALU.is_equal, fill=0.0,
            base=0, channel_multiplier=-1, pattern=[[1, 128]],
        )
        # triangular mask (1 where q_local >= k_partition) built once
        trimask = pool.tile([128, 128], BF16, tag="trimask", name="trimask")
        nc.vector.memset(trimask, 1.0)
        nc.gpsimd.affine_select(
            out=trimask,
            in_=trimask,
            compare_op=ALU.is_ge,
            fill=0.0,
            base=0,
            channel_multiplier=-1,
            pattern=[[1, 128]],
        )

        # W DMAs round-robin over the SP and Pool queues (NEVER the ACT
        # queue: exp instructions must not wait behind DMA issuance there)
        _dmaq = [nc.sync, nc.gpsimd]
        _dmai = [0]

        def _dma(dst, src):
            q = _dmaq[_dmai[0] % len(_dmaq)]
            _dmai[0] += 1
            q.dma_start(dst, src)

        def wtile(name, dram):
            # [p, st, i, col]: W row = 256*st + 128*i + p; one 4-dim DMA
            t = pool.tile([128, 4, 2, WCOLS], FP8, tag=name, name=name)
            wsrc = bass.AP(
                tensor=dram.tensor,
                offset=dram.offset,
                ap=[[WCOLS, 128], [256 * WCOLS, 4], [128 * WCOLS, 2], [1, WCOLS]],
            )
            _dma(t, wsrc)
            return t

        # x DMAs for the t4-th 512-token slab (issued before most W loads so
        # the serial DMA resource delivers the qkv(0) dependencies first)
        xts: dict = {}

        def emit_x_dma(t4):
            xb = {}
            for nm, src in (("hi", xhi_d), ("lo", xlo_d)):
                xt = xpool.tile([128, 4, 2, 512], FP8, tag=f"xb{nm}", name=f"xb{nm}")
                xsrc = bass.AP(
                    tensor=src.tensor,
                    offset=src.offset + t4 * 512,
                    ap=[[T, 128], [256 * T, 4], [128 * T, 2], [1, 512]],
                )
                nc.sync.dma_start(xt, xsrc)
                xb[nm] = xt
            xts[t4] = xb

        emit_x_dma(0)
        # QK weights next: the attention stream depends on Q^T/K^T quarters;
        # V/proj weights are consumed later and load after.
        w_sb = {k: wtile(k, w_d[k]) for k in
                ("wqhi", "wqlo", "wkhi", "wklo", "wvhi", "wvlo")}
        wpt = pool.tile([128, NPAIR, C], BF16, tag="wpb", name="wpb")
        wpsrc = bass.AP(
            tensor=wp.tensor, offset=wp.offset, ap=[[C, 128], [128 * C, NPAIR], [1, C]]
        )
        _dma(wpt, wpsrc)
        wp_sb = [wpt[:, p, :] for p in range(NPAIR)]

        if use_bias:
            bq_sb = pool.tile([128, 4], F32)
            bk_sb = pool.tile([128, 4], F32)
            nc.sync.dma_start(bq_sb, bq)
            nc.sync.dma_start(bk_sb, bk)
            bv_sb = pool.tile([128, WCOLS], F32)
            bv_bcast = bass.AP(
                tensor=bv.tensor, offset=bv.offset, ap=[[0, 128], *bv.ap]
            )
            nc.sync.dma_start(bv_sb, bv_bcast)

        # ====== fully interleaved pipeline over 512-token slabs ======

        def emit_dr3_st(ps, lhs_of_st, st):
            """3 DoubleRow matmuls (one contraction step st of 4):
            xhi@Whi + xhi@Wlo + xlo@Whi."""
            for j, (xk, wk_) in enumerate((("hi", "hi"), ("hi", "lo"), ("lo", "hi"))):
                nc.tensor.matmul(
                    ps,
                    lhsT=lhs_of_st(st, xk, wk_)[0],
                    rhs=lhs_of_st(st, xk, wk_)[1],
                    start=(st == 0 and j == 0),
                    stop=(st == 3 and j == 2),
                    perf_mode=DR,
                )

        def qkv_groups(t4):
            """Fine-grained thunks for slab t4: per psum group, one thunk per
            contraction step (3 DoubleRow mms each) plus an evac thunk, so
            injection never inserts a long PE burst ahead of an S matmul."""
            xb = xts[t4]
            thunks = []
            state = {}

            def v_mm(tt, st):
                if st == 0:
                    state[("v", tt)] = ps_mm.tile([128, 512], F32, tag="ps", name="ps")
                emit_dr3_st(
                    state[("v", tt)],
                    lambda st_, xk, wk_, tt=tt: (
                        xb[xk][:, st_, :, tt * 128 : (tt + 1) * 128],
                        w_sb["wv" + wk_][:, st_, :, :],
                    ),
                    st,
                )

            def v_evac(tt):
                kci = t4 * 4 + tt
                ps = state.pop(("v", tt))
                psv = ps.rearrange("p (h d) -> p h d", h=HPC)
                if use_bias:
                    nc.vector.scalar_tensor_tensor(
                        out=V[kci][:, :, 0:DH],
                        in0=psv,
                        scalar=1.0 / WS,
                        in1=bv_sb.rearrange("p (h d) -> p h d", h=HPC),
                        op0=ALU.mult,
                        op1=ALU.add,
                    )
                else:
                    nc.vector.tensor_scalar(
                        out=V[kci][:, :, 0:DH], in0=psv,
                        scalar1=1.0 / WS, scalar2=None, op0=ALU.mult,
                    )

            def qk_mm(wname, qt, st):
                if st == 0:
                    state[(wname, qt)] = ps_mm.tile([128, 512], F32, tag="ps", name="ps")
                emit_dr3_st(
                    state[(wname, qt)],
                    lambda st_, xk, wk_, qt=qt, wname=wname: (
                        w_sb[wname + wk_][:, st_, :, qt * 128 : (qt + 1) * 128],
                        xb[xk][:, st_, :, :],
                    ),
                    st,
                )

            def qk_evac(wname, dstf, bias, qt):
                ps = state.pop((wname, qt))
                dst = dstf[t4][:, qt // 2, qt % 2, :]
                if use_bias:
                    bsb = bq_sb if bias == "bq" else bk_sb
                    nc.vector.tensor_scalar(
                        out=dst, in0=ps,
                        scalar1=bsb[:, qt : qt + 1], scalar2=None, op0=ALU.add,
                    )
                else:
                    nc.vector.tensor_copy(dst, ps)

            # pair-0 of the next attention slab needs quarters 0,1 of both
            # Q and K first; emit quarter-major interleaving wq/wk.
            for qt in range(4):
                for wname, dstf, bias in (("wq", QTf, "bq"), ("wk", KTf, "bk")):
                    for st in range(4):
                        thunks.append(
                            lambda wname=wname, qt=qt, st=st: qk_mm(wname, qt, st)
                        )
                    thunks.append(
                        lambda wname=wname, dstf=dstf, bias=bias, qt=qt:
                        qk_evac(wname, dstf, bias, qt)
                    )
            for tt in range(4):
                for st in range(4):
                    thunks.append(lambda tt=tt, st=st: v_mm(tt, st))
                thunks.append(lambda tt=tt: v_evac(tt))
            thunks.append(lambda: xts.pop(t4, None))
            return thunks

        def emit_attention(p, q, inject=None):
            qsl = slice(q * 512, (q + 1) * 512)
            nblk = 4 * q + 4
            # q-major Y: per 128-token q-chunk lc and head hloc, accumulate
            # y[q, d] (+ Z in col 64 via V's ones column) over k-blocks.
            # Two PSUM banks hold (lc 0,1) and (lc 2,3); the four slices in a
            # bank are separate accumulation groups, so only the very first
            # matmul into the bank carries start=True (its bank-wide
            # pending-zero mark lets every slice's first touch overwrite).
            yA = ps_y.tile([128, 2, 2, 65], F32, tag="ya", name="ya")
            yB = ps_y.tile([128, 2, 2, 65], F32, tag="yb", name="yb")
            banks = (yA, yB)
            def emit_y(k, d, pt):
                lc0 = max(0, k - 4 * q)
                for lc in range(lc0, 4):
                    for hloc in (0, 1):
                        nc.tensor.matmul(
                            banks[lc // 2][:, lc % 2, hloc, :],
                            lhsT=pt[:, hloc * 512 + lc * 128 : hloc * 512 + (lc + 1) * 128],
                            rhs=V[k][:, 2 * p + hloc, :],
                            start=(k == 0 and lc % 2 == 0 and hloc == 0),
                            stop=(k == 4 * q + lc),
                            skip_group_check=True,
                        )

            pending = []  # two-block software skew: Y(k-2) after S(k), so a
            # stalled Y never blocks the next S (and thus exp) in PE order
            for k in range(nblk):
                # diagonal offset: columns q < d of this block are
                # fully masked -> restrict all work to q >= d
                d = max(0, 128 * k - 512 * q)
                # S^T block [128 k, 512-d q], DoubleRow fp8, d=64 contraction
                s = ps_s.tile([128, 1024], F32, tag="s", name="s")
                for hloc in (0, 1):
                    h = 2 * p + hloc
                    bp = 32 * (h % 4)
                    hh = h // 4
                    nc.tensor.matmul(
                        s[:, hloc * 512 + d : (hloc + 1) * 512],
                        lhsT=KTf[k // 4][bp : bp + 32, hh, :,
                                         (k % 4) * 128 : (k % 4 + 1) * 128],
                        rhs=QTf[q][bp : bp + 32, hh, :, d:512],
                        start=True,
                        stop=True,
                        perf_mode=DR,
                        tile_position=(bp, 0),
                    )
                if len(pending) >= 1:
                    emit_y(*pending.pop(0))
                pt = ptpool.tile([128, 1024], BF16, tag="pt", name="pt")
                ptv = pt.rearrange("p (h q) -> p h q", h=2)
                sv = s.rearrange("p (h q) -> p h q", h=2)
                nc.scalar.activation(
                    ptv[:, :, d:512], sv[:, :, d:512], AF.Exp,
                    scale=SCALE / (WS * WS),
                )
                if k >= 4 * q:
                    # triangular boundary band: zero where q_b < k, done on
                    # the otherwise-idle gpsimd engine (SBUF-only op)
                    nc.gpsimd.affine_select(
                        out=ptv[:, :, d : d + 128],
                        in_=ptv[:, :, d : d + 128],
                        compare_op=ALU.is_ge,
                        fill=0.0,
                        base=0,
                        channel_multiplier=-1,
                        pattern=[[0, 2], [1, 128]],
                    )
                pending.append((k, d, pt))
                if inject is not None:
                    inject(1)
            for pe_ in pending:
                emit_y(*pe_)
            # normalize per q-partition (Z sits in col 64 of each slice),
            # then PE-transpose back to feature-major rows of YT[p]
            ysbs = []
            for yb_ in banks:
                zr = zpool.tile([128, 2, 2, 1], F32, tag="zr", name="zr")
                nc.vector.reciprocal(out=zr, in_=yb_[:, :, :, 64:65])
                ysb = ytmpool.tile([128, 2, 2, DH], BF16, tag="ysb", name="ysb")
                zrb = bass.AP(
                    tensor=zr.tensor, offset=zr.offset,
                    ap=[zr.ap[0], zr.ap[1], zr.ap[2], [0, DH]],
                )
                nc.vector.tensor_tensor(
                    out=ysb, in0=yb_[:, :, :, 0:DH], in1=zrb, op=ALU.mult
                )
                ysbs.append(ysb)
            pst = ps_mm.tile([128, 4, 128], BF16, tag="ps", name="pst")
            for lc in range(4):
                nc.tensor.transpose(
                    pst[:, lc, :], ysbs[lc // 2][:, lc % 2, :, :], ident
                )
            nc.vector.tensor_copy(YT[p][:, qsl], pst.rearrange("p a b -> p (a b)"))

        def emit_proj(tt):
            o = opool.tile([128, 1024], BF16, tag="o", name="o")
            for n2 in range(2):
                nsl = slice(n2 * 512, (n2 + 1) * 512)
                ps = ps_mm.tile([128, 512], F32, tag="ps", name="ps")
                for p in range(NPAIR):
                    nc.tensor.matmul(
                        ps,
                        lhsT=YT[p][:, tt * 128 : (tt + 1) * 128],
                        rhs=wp_sb[p][:, nsl],
                        start=(p == 0),
                        stop=(p == NPAIR - 1),
                    )
                nc.vector.tensor_copy(o[:, nsl], ps)
            nc.gpsimd.dma_start(out[tt * 128 : (tt + 1) * 128, :], o)

        def proj_groups(q):
            return [lambda tt=tt: emit_proj(tt) for tt in range(4 * q, 4 * q + 4)]

        # software-pipelined schedule: slab t4's qkv matmul groups (and the
        # previous slab's projection) are injected between attention kblocks
        # of slab t4-1 so the PE has ready work while ACT chews on exp.
        for g in qkv_groups(0):
            g()
        for t4 in range(1, QC + 1):
            thunks = []
            if t4 < QC:
                emit_x_dma(t4)
                thunks += qkv_groups(t4)
            if t4 >= 2:
                thunks += proj_groups(t4 - 2)
            it = iter(thunks)
            slots = NPAIR * (4 * (t4 - 1) + 4)
            nt = len(thunks)
            ctr = [0, 0]  # slots seen, thunks fired

            def inject(n):
                ctr[0] += n
                target = min(nt, (ctr[0] * nt + slots - 1) // slots)
                while ctr[1] < target:
                    ctr[1] += 1
                    g = next(it, None)
                    if g is not None:
                        g()

            for p in range(NPAIR):
                emit_attention(p, t4 - 1, inject=inject)
            for g in it:
                g()
        for g in proj_groups(QC - 1):
            g()

    nc.compile()
    return nc


_PROGRAMS: dict = {}


def _get_program(use_bias: bool):
    if use_bias not in _PROGRAMS:
        _PROGRAMS[use_bias] = _build_program(use_bias)
    return _PROGRAMS[use_bias]


def _bf16(a):
    return np.ascontiguousarray(a.astype(ml_dtypes.bfloat16))


def _fp8_hilo(a32):
    """a32 (f32, already x32-scaled) -> (hi, lo) e4m3 with lo = residual."""
    hi = a32.astype(E4)
    lo = (a32 - hi.astype(np.float32)).astype(E4)
    return np.ascontiguousarray(hi), np.ascontiguousarray(lo)


# fold-layout column map for Q^T/K^T quarters: quarter qt=(hh,dhi), row j
_COLMAP = np.empty(512, dtype=np.int64)
for _qt in range(4):
    _hh, _dhi = _qt // 2, _qt % 2
    _j = np.arange(128)
    _COLMAP[_qt * 128 + _j] = 64 * (4 * _hh + _j // 32) + 32 * _dhi + (_j % 32)


def kernel(x, W_qkv, b_qkv, W_proj, b_proj):
    x = np.asarray(x, dtype=np.float32)
    W_qkv = np.asarray(W_qkv, dtype=np.float32)
    b_qkv = np.asarray(b_qkv, dtype=np.float32)
    W_proj = np.asarray(W_proj, dtype=np.float32)
    b_proj = np.asarray(b_proj, dtype=np.float32)

    use_bias = bool(np.any(b_qkv != 0.0))
    nc = _get_program(use_bias)

    # per-batch x splits (shared by the 2 cores of each batch)
    x8 = []
    for b in range(B):
        xT = np.ascontiguousarray(x[b].T)
        hi, lo = _fp8_hilo(xT)  # x stored at true scale
        x8.append({"xhi": hi, "xlo": lo})

    shard = []
    for s in range(2):
        m = {}
        for ti, tname in enumerate(("wq", "wk", "wv")):
            w = W_qkv[:, ti * C + s * WCOLS : ti * C + (s + 1) * WCOLS]
            if tname != "wv":
                w = w[:, _COLMAP]
            hi, lo = _fp8_hilo(w * WS)
            m[tname + "hi"], m[tname + "lo"] = hi, lo
        m["wp"] = _bf16(W_proj[s * WCOLS : (s + 1) * WCOLS, :])
        if use_bias:
            bq_s = b_qkv[s * WCOLS : (s + 1) * WCOLS]
            bk_s = b_qkv[C + s * WCOLS : C + (s + 1) * WCOLS]
            m["bq"] = np.ascontiguousarray(
                (WS * bq_s[_COLMAP]).reshape(4, 128).T.astype(np.float32)
            )
            m["bk"] = np.ascontiguousarray(
                (WS * bk_s[_COLMAP]).reshape(4, 128).T.astype(np.float32)
            )
            m["bv"] = np.ascontiguousarray(
                b_qkv[2 * C + s * WCOLS : 2 * C + (s + 1) * WCOLS]
            )
        shard.append(m)

    in_maps = [{**x8[c // 2], **shard[c % 2]} for c in range(NCORES)]

    res = run_bass_kernel_spmd(nc, in_maps, list(range(NCORES))).results

    outp = np.empty((B, T, C), dtype=np.float32)
    for b in range(B):
        outp[b] = res[2 * b]["out"].astype(np.float32) + res[2 * b + 1]["out"].astype(
            np.float32
        )
    outp += b_proj
    return outp


def modeled_ns(use_bias: bool = False) -> float:
    """Single-core cost-model estimate of the kernel duration."""
    from concourse.timeline_sim import TimelineSim

    return TimelineSim(_build_program(use_bias)).simulate()
